# revision 1
# baseline (speedup 1.0000x reference)
"""Multi-head self-attention (B=8, T=2048, C=192, H=6, HS=32) on 8 TRN2 cores.

Sharding: data-parallel over batch — core i computes batch element i fully
on-chip (no collectives). Host pre-transposes x and packs weights so the
device does zero transposes:

  qT/kT [d, t] = Wq_packed.T @ x.T          (d = h*HS + dd)
  v     [s, d] = x @ Wv_packed, stored per-head as [v_h | ones] (33 cols)
  S^T   [s, t] = kT_h.T @ qT_h              (K=32 matmuls, row-group packed)
  P^T          = exp(S^T / sqrt(HS))        (ScalarE, PSUM->SBUF, bf16)
  [O^T_h; rowsum_h x32] = [v_h|1x32].T @ P^T  (rowsum replicated to rows 32-63)
  OTn_h [d, t] = O^T_h * (1/rowsum_h)       (DVE reciprocal + mul, no bcast)
  out   [t, c] = sum_h OTn_h.T @ Wproj_h + bias  (K=32 accum + rank-1 bias)
"""

import numpy as np
import ml_dtypes
from contextlib import ExitStack

import concourse.bass as bass
import concourse.tile as tile
from concourse import bacc, mybir
from concourse.bass_utils import run_bass_kernel_spmd

B, T, C = 8, 2048, 192
H, HS = 6, 32
P = 128
TCH = 512            # t-chunk width (one PSUM bank of fp32)
NT = T // TCH        # 4
NS = T // P          # 16 s-tiles
SCALE = 1.0 / float(np.sqrt(HS))
BF16 = mybir.dt.bfloat16
F32 = mybir.dt.float32
Exp = mybir.ActivationFunctionType.Exp

_CACHE = {}


def build_nc():
    nc = bacc.Bacc()
    xT = nc.declare_dram_parameter("xT", [C, T], BF16, isOutput=False)
    wq = nc.declare_dram_parameter("wq", [C, H * HS], BF16, isOutput=False)
    wk = nc.declare_dram_parameter("wk", [C, H * HS], BF16, isOutput=False)
    wv = nc.declare_dram_parameter("wv", [C, H * HS], BF16, isOutput=False)
    wp = nc.declare_dram_parameter("wp", [H, HS, C], BF16, isOutput=False)
    bp = nc.declare_dram_parameter("bp", [1, C], BF16, isOutput=False)
    out = nc.declare_dram_parameter("out", [T, C], F32, isOutput=True)

    with tile.TileContext(nc) as tc, ExitStack() as ctx:
        singles = ctx.enter_context(tc.tile_pool(name="singles", bufs=1))
        qk_pool = ctx.enter_context(tc.tile_pool(name="qk", bufs=1))
        vaug_pool = ctx.enter_context(tc.tile_pool(name="vaug", bufs=1))
        pt_pool = ctx.enter_context(tc.tile_pool(name="ptp", bufs=4))
        otn_pool = ctx.enter_context(tc.tile_pool(name="otn", bufs=1))
        small = ctx.enter_context(tc.tile_pool(name="small", bufs=4))
        ysb_pool = ctx.enter_context(tc.tile_pool(name="ysb", bufs=3))

        # ---------------- load inputs ----------------
        xT_a = singles.tile([P, T], BF16)
        nc.sync.dma_start(xT_a, xT[0:P, :])
        xT_b = singles.tile([C - P, T], BF16)
        nc.sync.dma_start(xT_b, xT[P:C, :])

        w_sb = {}
        for name, dram in (("q", wq), ("k", wk), ("v", wv)):
            a = singles.tile([P, H * HS], BF16, name=f"w{name}a")
            nc.sync.dma_start(a, dram[0:P, :])
            b = singles.tile([C - P, H * HS], BF16, name=f"w{name}b")
            nc.sync.dma_start(b, dram[P:C, :])
            w_sb[name] = (a, b)

        wp_sb = []
        for h in range(H):
            wph = singles.tile([HS, C], BF16, name=f"wp{h}")
            nc.sync.dma_start(wph, wp[h, :, :])
            wp_sb.append(wph)
        bp_sb = singles.tile([1, C], BF16)
        nc.sync.dma_start(bp_sb, bp[:, :])
        ones1 = singles.tile([1, P], BF16)
        nc.vector.memset(ones1, 1.0)

        # ---------------- phase 1: qT, kT, v_aug ----------------
        qT_a = qk_pool.tile([P, T], BF16)       # heads 0..3, d-major
        qT_b = qk_pool.tile([C - P, T], BF16)   # heads 4,5
        kT_a = qk_pool.tile([P, T], BF16)
        kT_b = qk_pool.tile([C - P, T], BF16)
        v_aug = []
        with tc.tile_pool(name="pqkv", bufs=2, space="PSUM") as pqkv:
            for proj, dst_a, dst_b in (("q", qT_a, qT_b), ("k", kT_a, kT_b)):
                wa, wb = w_sb[proj]
                for dlo, dsz, dst in ((0, P, dst_a), (P, C - P, dst_b)):
                    for t0 in range(0, T, TCH):
                        ps = pqkv.tile([P, TCH], F32, name="psq", tag="psq")
                        nc.tensor.matmul(
                            ps[0:dsz, :], wa[:, dlo:dlo + dsz],
                            xT_a[:, t0:t0 + TCH], start=True, stop=False)
                        nc.tensor.matmul(
                            ps[0:dsz, :], wb[:, dlo:dlo + dsz],
                            xT_b[:, t0:t0 + TCH], start=False, stop=True)
                        nc.vector.tensor_copy(
                            dst[0:dsz, t0:t0 + TCH], ps[0:dsz, :])
            wva, wvb = w_sb["v"]
            for si in range(NS):
                s0 = si * P
                va = vaug_pool.tile(
                    [P, H * 2 * HS], BF16, name=f"vaug{si}", tag=f"vaug{si}")
                ps = pqkv.tile([P, H * HS], F32, name="psv", tag="psv")
                nc.tensor.matmul(ps, xT_a[:, s0:s0 + P], wva,
                                 start=True, stop=False)
                nc.tensor.matmul(ps, xT_b[:, s0:s0 + P], wvb,
                                 start=False, stop=True)
                va_r = va.rearrange("p (h e) -> p h e", h=H)
                ps_r = ps.rearrange("p (h d) -> p h d", h=H)
                nc.vector.tensor_copy(va_r[:, :, 0:HS], ps_r)
                nc.vector.memset(va_r[:, :, HS:2 * HS], 1.0)
                v_aug.append(va)

        # ---------------- phase 2: attention ----------------
        otn = [otn_pool.tile([HS, T], BF16, name=f"otn{h}", tag=f"otn{h}")
               for h in range(H)]
        # head pairs (A=2p, B=2p+1); within a pair kT/qT rows sit in
        # distinct 32-row groups, so the two QKT matmuls run concurrently
        def hsrc(h):
            if h < 4:
                return kT_a, qT_a, HS * h
            return kT_b, qT_b, HS * (h - 4)
        with (
            tc.tile_pool(name="pst", bufs=2, space="PSUM") as pst_pool,
            tc.tile_pool(name="pav", bufs=1, space="PSUM") as pav_pool,
            tc.tile_pool(name="py", bufs=1, space="PSUM") as py_pool,
        ):
            for tc0 in range(0, T, TCH):
                av = [pav_pool.tile([P, TCH], F32,
                                    name=f"avp{p}", tag=f"avp{p}")
                      for p in range(H // 2)]
                for si in range(NS):
                    s0 = si * P
                    for p in range(H // 2):
                        hA, hB = 2 * p, 2 * p + 1
                        stp = pst_pool.tile([P, 2 * TCH], F32,
                                            name="stp", tag="stp")
                        for half, h in ((0, hA), (1, hB)):
                            kT_t, qT_t, pb = hsrc(h)
                            nc.tensor.matmul(
                                stp[:, half * TCH:(half + 1) * TCH],
                                kT_t[pb:pb + HS, s0:s0 + P],
                                qT_t[pb:pb + HS, tc0:tc0 + TCH],
                                start=True, stop=True, tile_position=(pb, 0))
                        ptp = pt_pool.tile([P, 2 * TCH], BF16,
                                           name="ptp", tag="ptp")
                        nc.scalar.activation(ptp, stp, Exp, scale=SCALE)
                        for half, h in ((0, hA), (1, hB)):
                            nc.tensor.matmul(
                                av[p][64 * half:64 * half + 64, :],
                                v_aug[si][:, 2 * HS * h:2 * HS * (h + 1)],
                                ptp[:, half * TCH:(half + 1) * TCH],
                                start=(si == 0), stop=(si == NS - 1),
                                skip_group_check=True,
                                tile_position=(0, 64 * half))
                for p in range(H // 2):
                    rbp = small.tile([P, TCH], F32, name="rbp", tag="rbp")
                    for half, h in ((0, 2 * p), (1, 2 * p + 1)):
                        b = 64 * half
                        nc.vector.reciprocal(
                            rbp[b:b + HS, :], av[p][b + HS:b + 2 * HS, :])
                        nc.vector.tensor_mul(
                            otn[h][:, tc0:tc0 + TCH],
                            av[p][b:b + HS, :], rbp[b:b + HS, :])
                # ---- projection for this t-chunk (spare PSUM bank) ----
                for tt in range(tc0, tc0 + TCH, P):
                    ps = py_pool.tile([P, C], F32, name="psy", tag="psy")
                    nc.tensor.matmul(ps, ones1, bp_sb, start=True, stop=False)
                    for h in range(H):
                        nc.tensor.matmul(
                            ps, otn[h][:, tt:tt + P], wp_sb[h],
                            start=False, stop=(h == H - 1))
                    ysb = ysb_pool.tile([P, C], F32, name="ysbt", tag="ysbt")
                    nc.vector.tensor_copy(ysb, ps)
                    nc.sync.dma_start(out[tt:tt + P, :], ysb)

    nc.compile()
    return nc


def _get_nc():
    if "nc" not in _CACHE:
        _CACHE["nc"] = build_nc()
    return _CACHE["nc"]


def make_in_maps(x, Wq, Wk, Wv, Wproj, bproj):
    bf = ml_dtypes.bfloat16
    x = np.asarray(x, np.float32)
    pack = lambda w: np.ascontiguousarray(
        np.transpose(np.asarray(w, np.float32), (1, 0, 2)).reshape(C, H * HS)
    ).astype(bf)
    wq, wk, wv = pack(Wq), pack(Wk), pack(Wv)
    wp = np.ascontiguousarray(
        np.asarray(Wproj, np.float32).reshape(H, HS, C)).astype(bf)
    bp = np.asarray(bproj, np.float32).reshape(1, C).astype(bf)
    maps = []
    for i in range(B):
        xti = np.ascontiguousarray(x[i].T).astype(bf)
        maps.append({"xT": xti, "wq": wq, "wk": wk, "wv": wv,
                     "wp": wp, "bp": bp})
    return maps


def run(inputs, trace=False, **kw):
    nc = _get_nc()
    in_maps = make_in_maps(**inputs)
    res = run_bass_kernel_spmd(nc, in_maps, core_ids=list(range(B)),
                               trace=trace, **kw)
    y = np.stack([np.asarray(res.results[i]["out"], np.float32)
                  for i in range(B)], axis=0)
    return y, res


def kernel(**inputs):
    y, _ = run(inputs, trace=False)
    return y



# revision 7
# speedup vs baseline: 1.1924x; 1.1924x over previous
"""Multi-head self-attention (B=8, T=2048, C=192, H=6, HS=32) on 8 TRN2 cores.

Data-parallel over batch: core i computes batch element i fully on-chip.

Design (driven by the CoreSim cost model, which charges a matmul only its
streamed output columns):
  qT/kT [d,t] kept fp32 (float32r matmuls: 1 cyc/row at N>=512) - exact scores.
  S^T [s,t] tiles per head pair -> exp split across ACT (exact) / DVE / GpSimd
  (Schraudolph int16 bit-trick writing bf16 bit patterns directly).
  AV flipped: O[t,d] = P^T[s,t-tile].T @ [v_h | 1]  (N=33 streamed cols; the
  ones column accumulates the softmax denominator r as col 32).
  Normalize with r on partitions (reciprocal + one broadcast multiply), PE
  transpose [t,d]->[d,t], then the output projection with the bias folded in
  as a ones row of otn_b.
"""

import numpy as np
import ml_dtypes
from contextlib import ExitStack

import concourse.bass as bass
import concourse.tile as tile
from concourse import bacc, mybir
from concourse.bass import broadcast_tensor_aps
from concourse.bass_utils import run_bass_kernel_spmd

B, T, C = 8, 2048, 192
H, HS = 6, 32
P = 128
TCH = 512            # t-chunk per tc0 block
NT = T // TCH        # 4
NS = T // P          # 16 s-tiles
NJ = TCH // P        # 4 t-subtiles per chunk
SCALE = 1.0 / float(np.sqrt(HS))
BF16 = mybir.dt.bfloat16
F32 = mybir.dt.float32
F32R = mybir.dt.float32r
I16 = mybir.dt.int16
Exp = mybir.ActivationFunctionType.Exp

# Schraudolph constants for bf16-domain exp: bits = int16(s*EXP_A + EXP_B),
# reinterpreted as bf16 ~= exp(s*SCALE).
EXP_A = SCALE * 128.0 / float(np.log(2.0))
EXP_B = 127.0 * 128.0 - 486411.0 / 65536.0 + 0.5

_CACHE = {}

# engine assignment for exp tiles: index = si*3+p (48 per tc0)
# A=ACT exact exp, D=DVE Schraudolph. GpSimd cannot access PSUM.
N_ACT = 27  # of 48 per tc0


def _exp_engine(idx):
    if idx < 3:
        return "A"
    return "A" if (idx * N_ACT) // 48 != ((idx + 1) * N_ACT) // 48 else "D"


_EXPPAT = "".join(_exp_engine(i) for i in range(48))


def build_nc():
    nc = bacc.Bacc()
    xT = nc.declare_dram_parameter("xT", [C, T], F32R, isOutput=False)
    wq = nc.declare_dram_parameter("wq", [C, H * HS], F32R, isOutput=False)
    wk = nc.declare_dram_parameter("wk", [C, H * HS], F32R, isOutput=False)
    x16 = nc.declare_dram_parameter("x16", [C, T], BF16, isOutput=False)
    wv = nc.declare_dram_parameter("wv", [C, H * HS], BF16, isOutput=False)
    wpa = nc.declare_dram_parameter("wpa", [P, C], BF16, isOutput=False)
    wpb = nc.declare_dram_parameter("wpb", [C - P + 1, C], BF16, isOutput=False)
    ident = nc.declare_dram_parameter("ident", [P, P], BF16, isOutput=False)
    out = nc.declare_dram_parameter("out", [T, C], F32, isOutput=True)

    with tile.TileContext(nc) as tc, ExitStack() as ctx:
        singles = ctx.enter_context(tc.tile_pool(name="singles", bufs=1))
        vpool = ctx.enter_context(tc.tile_pool(name="vpool", bufs=1))
        ptp_pool = ctx.enter_context(tc.tile_pool(name="ptp", bufs=6))
        stage_pool = ctx.enter_context(tc.tile_pool(name="stage", bufs=2))
        rr_pool = ctx.enter_context(tc.tile_pool(name="rr", bufs=2))
        ysb_pool = ctx.enter_context(tc.tile_pool(name="ysb", bufs=3))

        # ---------------- input DMA ----------------
        xT_a = singles.tile([P, T], F32R)
        xT_b = singles.tile([C - P, T], F32R)
        x16_a = singles.tile([P, T], BF16)
        x16_b = singles.tile([C - P, T], BF16)
        for t0 in range(0, T, TCH):
            nc.sync.dma_start(xT_a[:, t0:t0 + TCH], xT[0:P, t0:t0 + TCH])
            nc.sync.dma_start(xT_b[:, t0:t0 + TCH], xT[P:C, t0:t0 + TCH])
            nc.sync.dma_start(x16_a[:, t0:t0 + TCH], x16[0:P, t0:t0 + TCH])
            nc.sync.dma_start(x16_b[:, t0:t0 + TCH], x16[P:C, t0:t0 + TCH])
        wq_a = singles.tile([P, H * HS], F32R)
        wq_b = singles.tile([C - P, H * HS], F32R)
        wk_a = singles.tile([P, H * HS], F32R)
        wk_b = singles.tile([C - P, H * HS], F32R)
        wv_a = singles.tile([P, H * HS], BF16)
        wv_b = singles.tile([C - P, H * HS], BF16)
        nc.sync.dma_start(wq_a, wq[0:P, :])
        nc.sync.dma_start(wq_b, wq[P:C, :])
        nc.sync.dma_start(wk_a, wk[0:P, :])
        nc.sync.dma_start(wk_b, wk[P:C, :])
        nc.sync.dma_start(wv_a, wv[0:P, :])
        nc.sync.dma_start(wv_b, wv[P:C, :])
        wpa_sb = singles.tile([P, C], BF16)
        wpb_sb = singles.tile([C - P + 1, C], BF16)
        id_sb = singles.tile([P, P], BF16)
        nc.sync.dma_start(wpa_sb, wpa[:, :])
        nc.sync.dma_start(wpb_sb, wpb[:, :])
        nc.sync.dma_start(id_sb, ident[:, :])

        qT_a = singles.tile([P, T], F32R)
        qT_b = singles.tile([C - P, T], F32R)
        kT_a = singles.tile([P, T], F32R)
        kT_b = singles.tile([C - P, T], F32R)
        otn_a = singles.tile([P, T], BF16)
        otn_b = singles.tile([C - P + 1, T], BF16)
        # ones row for the bias trick in the output projection
        nc.gpsimd.memset(otn_b[C - P:C - P + 1, :], 1.0)

        v33 = []
        for si in range(NS):
            v33.append(vpool.tile([P, H * 33], BF16, name=f"v33_{si}"))

        # ---------------- phase 1: qT, kT (fp32), v33 (bf16) ----------------
        with (
            tc.tile_pool(name="pq", bufs=2, space="PSUM") as pq,
            tc.tile_pool(name="pv", bufs=2, space="PSUM") as pv,
        ):
            for t0 in range(0, T, TCH):
                for name, wa, wb, dsta, dstb in (
                    ("q", wq_a, wq_b, qT_a, qT_b),
                    ("k", wk_a, wk_b, kT_a, kT_b),
                ):
                    for dlo, dsz, dst in ((0, P, dsta), (P, C - P, dstb)):
                        ps = pq.tile([P, TCH], F32, name="psq", tag="psq")
                        nc.tensor.matmul(
                            ps[0:dsz, :], wa[:, dlo:dlo + dsz],
                            xT_a[:, t0:t0 + TCH],
                            start=True, stop=False)
                        nc.tensor.matmul(
                            ps[0:dsz, :], wb[:, dlo:dlo + dsz],
                            xT_b[:, t0:t0 + TCH],
                            start=False, stop=True)
                        if name == "q":
                            nc.scalar.copy(dst[0:dsz, t0:t0 + TCH], ps[0:dsz, :])
                        else:
                            nc.vector.tensor_copy(
                                dst[0:dsz, t0:t0 + TCH], ps[0:dsz, :])
                for si in range(t0 // P, t0 // P + NJ):
                    s0 = si * P
                    ps = pv.tile([P, H * HS], F32, name="psv", tag="psv")
                    nc.tensor.matmul(ps, x16_a[:, s0:s0 + P],
                                     wv_a, start=True, stop=False)
                    nc.tensor.matmul(ps, x16_b[:, s0:s0 + P],
                                     wv_b, start=False, stop=True)
                    va_r = v33[si].rearrange("p (h e) -> p h e", h=H)
                    ps_r = ps.rearrange("p (h d) -> p h d", h=H)
                    nc.vector.tensor_copy(va_r[:, :, 0:HS], ps_r)
                    nc.gpsimd.memset(va_r[:, :, HS:HS + 1], 1.0)

        # ---------------- phase 2 ----------------
        def hsrc(h):
            if h < 4:
                return kT_a, qT_a, HS * h
            return kT_b, qT_b, HS * (h - 4)

        with (
            tc.tile_pool(name="pst", bufs=2, space="PSUM") as pst_pool,
            tc.tile_pool(name="pav", bufs=1, space="PSUM") as pav_pool,
            tc.tile_pool(name="pot", bufs=1, space="PSUM") as pot_pool,
            tc.tile_pool(name="py", bufs=1, space="PSUM") as py_pool,
        ):
            state = {}

            def emit_qkt(tci, si, p):
                tc0 = tci * TCH
                s0 = si * P
                stp = pst_pool.tile([P, 2 * TCH], F32, name="stp", tag="stp")
                for half in (0, 1):
                    h = 2 * p + half
                    kT_t, qT_t, pb = hsrc(h)
                    nc.tensor.matmul(
                        stp[:, half * TCH:(half + 1) * TCH],
                        kT_t[pb:pb + HS, s0:s0 + P],
                        qT_t[pb:pb + HS, tc0:tc0 + TCH],
                        start=True, stop=True, tile_position=(pb, 0))
                return stp

            def emit_exp(tci, si, p, stp):
                eng = _EXPPAT[si * 3 + p]
                ptp = ptp_pool.tile([P, 2 * TCH], BF16, name="ptp", tag="ptp")
                if eng == "A":
                    nc.scalar.activation(ptp, stp, Exp, scale=SCALE)
                else:
                    nc.vector.tensor_scalar(
                        ptp.bitcast(I16), stp, EXP_A, EXP_B,
                        mybir.AluOpType.mult, mybir.AluOpType.add)
                return ptp

            def emit_av(tci, si, ptps):
                # 24 matmuls: out[t,d] accumulated over si into 2 PSUM banks
                av = state["av"]
                for h in range(H):
                    bank, hl = divmod(h, 3)
                    ptp = ptps[h // 2]
                    half = h % 2
                    for j in range(NJ):
                        off = hl * 132 + j * 33
                        nc.tensor.matmul(
                            av[bank][:, off:off + 33],
                            ptp[:, half * TCH + j * P:half * TCH + (j + 1) * P],
                            v33[si][:, h * 33:(h + 1) * 33],
                            start=(si == 0 and hl == 0 and j == 0),
                            stop=(si == NS - 1 and hl == 2 and j == NJ - 1),
                            skip_group_check=True)

            def tail_norm(tci):
                av = state["av"]
                sa = stage_pool.tile([P, NJ * P], BF16, name="sa", tag="sa")
                sb = stage_pool.tile([P, NJ * (C - P)], BF16,
                                     name="sb", tag="sb")
                sa_v = sa.rearrange("p (j h e) -> p h j e", j=NJ, h=4)
                sb_v = sb.rearrange("p (j h e) -> p h j e", j=NJ, h=2)
                outs = []
                for bank in range(2):
                    rr = rr_pool.tile([P, 12], F32, name=f"rr{bank}",
                                      tag=f"rr{bank}")
                    av_v = av[bank][:, 0:396].rearrange(
                        "p (hl j e) -> p hl j e", hl=3, j=NJ)
                    rr_v = rr.rearrange("p (hl j e) -> p hl j e", hl=3, j=NJ)
                    nc.vector.reciprocal(rr_v, av_v[:, :, :, 32:33])
                    outs.append((av_v[:, :, :, 0:HS], rr_v))
                (o1, r1), (o2, r2) = outs
                for dst, src, rsrc in (
                    (sa_v[:, 0:3], o1, r1),
                    (sa_v[:, 3:4], o2[:, 0:1], r2[:, 0:1]),
                    (sb_v[:, 0:2], o2[:, 1:3], r2[:, 1:3]),
                ):
                    s_b, r_b = broadcast_tensor_aps(src, rsrc)
                    nc.vector.tensor_tensor(dst, s_b, r_b,
                                            mybir.AluOpType.mult)
                state["stage"] = (sa, sb)

            def tail_transpose(tci):
                sa, sb = state["stage"]
                pot = pot_pool.tile([P, 2 * NJ * P], BF16, name="pot",
                                    tag="pot")
                for j in range(NJ):
                    nc.tensor.transpose(
                        pot[:, j * 2 * P:j * 2 * P + P],
                        sa[:, j * P:(j + 1) * P], id_sb)
                    nc.tensor.transpose(
                        pot[0:C - P, j * 2 * P + P:(j + 1) * 2 * P],
                        sb[:, j * (C - P):(j + 1) * (C - P)], id_sb)
                state["pot"] = pot

            def tail_otcopy(tci):
                pot = state["pot"]
                tc0 = tci * TCH
                pot_v = pot.rearrange("p (j two t) -> p j two t", j=NJ, two=2)
                dst_a = otn_a[:, tc0:tc0 + TCH].rearrange(
                    "p (j t) -> p j t", j=NJ)
                dst_b = otn_b[0:C - P, tc0:tc0 + TCH].rearrange(
                    "p (j t) -> p j t", j=NJ)
                nc.vector.tensor_copy(dst_a, pot_v[:, :, 0, :])
                nc.vector.tensor_copy(dst_b, pot_v[0:C - P, :, 1, :])

            def tail_proj(tci, j):
                tt = tci * TCH + j * P
                py = py_pool.tile([P, C], F32, name="py", tag="py")
                nc.tensor.matmul(py, otn_a[:, tt:tt + P], wpa_sb,
                                 start=True, stop=False)
                nc.tensor.matmul(py, otn_b[:, tt:tt + P], wpb_sb,
                                 start=False, stop=True)
                ysb = ysb_pool.tile([P, C], F32, name="ysb", tag="ysb")
                nc.scalar.copy(ysb, py)
                nc.sync.dma_start(out[tt:tt + P, :], ysb)

            prev = None
            for tci in range(NT):
                av = [pav_pool.tile([P, TCH], F32, name=f"av{b}",
                                    tag=f"av{b}") for b in range(2)]
                if prev is not None:
                    tail_norm(prev)
                state["av"] = av
                pend = None
                for si in range(NS):
                    stps = [emit_qkt(tci, si, p) for p in range(3)]
                    if si == 1 and prev is not None:
                        tail_transpose(prev)
                    ptps = [emit_exp(tci, si, p, stps[p]) for p in range(3)]
                    if pend is not None:
                        emit_av(tci, si - 1, pend)
                    if prev is not None:
                        if si == 2:
                            tail_otcopy(prev)
                        elif 3 <= si <= 6:
                            tail_proj(prev, si - 3)
                    pend = ptps
                emit_av(tci, NS - 1, pend)
                prev = tci
            tail_norm(prev)
            tail_transpose(prev)
            tail_otcopy(prev)
            for j in range(NJ):
                tail_proj(prev, j)

    nc.compile()
    return nc


def _get_nc():
    if "nc" not in _CACHE:
        _CACHE["nc"] = build_nc()
    return _CACHE["nc"]


def make_in_maps(x, Wq, Wk, Wv, Wproj, bproj):
    bf = ml_dtypes.bfloat16
    x = np.asarray(x, np.float32)
    pack32 = lambda w: np.ascontiguousarray(
        np.transpose(np.asarray(w, np.float32), (1, 0, 2)).reshape(C, H * HS))
    wq_, wk_ = pack32(Wq), pack32(Wk)
    wv_ = pack32(Wv).astype(bf)
    wp = np.asarray(Wproj, np.float32)
    wpa_ = np.ascontiguousarray(wp[0:P, :]).astype(bf)
    wpb_ = np.concatenate(
        [wp[P:, :], np.asarray(bproj, np.float32).reshape(1, C)], axis=0
    ).astype(bf)
    ident = np.eye(P, dtype=np.float32).astype(bf)
    maps = []
    for i in range(B):
        xti = np.ascontiguousarray(x[i].T)
        maps.append({"xT": xti, "x16": xti.astype(bf), "wq": wq_, "wk": wk_,
                     "wv": wv_, "wpa": wpa_, "wpb": wpb_, "ident": ident})
    return maps


def run(inputs, trace=False, **kw):
    nc = _get_nc()
    in_maps = make_in_maps(**inputs)
    res = run_bass_kernel_spmd(nc, in_maps, core_ids=list(range(B)),
                               trace=trace, **kw)
    y = np.stack([np.asarray(res.results[i]["out"], np.float32)
                  for i in range(B)], axis=0)
    return y, res


def kernel(**inputs):
    y, _ = run(inputs, trace=False)
    return y


# revision 8
# speedup vs baseline: 1.2355x; 1.0361x over previous
"""Multi-head self-attention (B=8, T=2048, C=192, H=6, HS=32) on 8 TRN2 cores.

Data-parallel over batch: core i computes batch element i fully on-chip.

Design (driven by the CoreSim cost model, which charges a matmul only its
streamed output columns):
  qT/kT [d,t] kept fp32 (float32r matmuls: 1 cyc/row at N>=512) - exact scores.
  S^T [s,t] tiles per head pair -> exp split across ACT (exact) / DVE / GpSimd
  (Schraudolph int16 bit-trick writing bf16 bit patterns directly).
  AV flipped: O[t,d] = P^T[s,t-tile].T @ [v_h | 1]  (N=33 streamed cols; the
  ones column accumulates the softmax denominator r as col 32).
  Normalize with r on partitions (reciprocal + one broadcast multiply), PE
  transpose [t,d]->[d,t], then the output projection with the bias folded in
  as a ones row of otn_b.
"""

import numpy as np
import ml_dtypes
from contextlib import ExitStack

import concourse.bass as bass
import concourse.tile as tile
from concourse import bacc, mybir
from concourse.bass import broadcast_tensor_aps
from concourse.bass_utils import run_bass_kernel_spmd

B, T, C = 8, 2048, 192
H, HS = 6, 32
P = 128
TCH = 512            # t-chunk per tc0 block
NT = T // TCH        # 4
NS = T // P          # 16 s-tiles
NJ = TCH // P        # 4 t-subtiles per chunk
SCALE = 1.0 / float(np.sqrt(HS))
BF16 = mybir.dt.bfloat16
F32 = mybir.dt.float32
F32R = mybir.dt.float32r
I16 = mybir.dt.int16
Exp = mybir.ActivationFunctionType.Exp

# Schraudolph constants for bf16-domain exp: bits = int16(s*EXP_A + EXP_B),
# reinterpreted as bf16 ~= exp(s*SCALE).
EXP_A = SCALE * 128.0 / float(np.log(2.0))
EXP_B = 127.0 * 128.0 - 486411.0 / 65536.0 + 0.5

_CACHE = {}

# engine assignment for exp tiles: index = si*3+p (48 per tc0)
# A=ACT exact exp, D=DVE Schraudolph. GpSimd cannot access PSUM.
N_ACT = 27  # of 48 per tc0


def _exp_engine(idx):
    if idx < 6:
        return "A"
    return "A" if (idx * N_ACT) // 48 != ((idx + 1) * N_ACT) // 48 else "D"


_EXPPAT = "".join(_exp_engine(i) for i in range(48))


def build_nc():
    nc = bacc.Bacc()
    xT = nc.declare_dram_parameter("xT", [C, T], F32R, isOutput=False)
    wq = nc.declare_dram_parameter("wq", [C, H * HS], F32R, isOutput=False)
    wk = nc.declare_dram_parameter("wk", [C, H * HS], F32R, isOutput=False)
    x16 = nc.declare_dram_parameter("x16", [C, T], BF16, isOutput=False)
    wv = nc.declare_dram_parameter("wv", [C, H * HS], BF16, isOutput=False)
    wpa = nc.declare_dram_parameter("wpa", [P, C], BF16, isOutput=False)
    wpb = nc.declare_dram_parameter("wpb", [C - P + 1, C], BF16, isOutput=False)
    ident = nc.declare_dram_parameter("ident", [P, P], BF16, isOutput=False)
    out = nc.declare_dram_parameter("out", [T, C], F32, isOutput=True)

    with tile.TileContext(nc) as tc, ExitStack() as ctx:
        singles = ctx.enter_context(tc.tile_pool(name="singles", bufs=1))
        vpool = ctx.enter_context(tc.tile_pool(name="vpool", bufs=1))
        ptp_pool = ctx.enter_context(tc.tile_pool(name="ptp", bufs=6))
        stage_pool = ctx.enter_context(tc.tile_pool(name="stage", bufs=2))
        rr_pool = ctx.enter_context(tc.tile_pool(name="rr", bufs=2))
        ysb_pool = ctx.enter_context(tc.tile_pool(name="ysb", bufs=3))

        # ---------------- input DMA ----------------
        xT_a = singles.tile([P, T], F32R)
        xT_b = singles.tile([C - P, T], F32R)
        x16_a = singles.tile([P, T], BF16)
        x16_b = singles.tile([C - P, T], BF16)
        wq_a = singles.tile([P, H * HS], F32R)
        wq_b = singles.tile([C - P, H * HS], F32R)
        wk_a = singles.tile([P, H * HS], F32R)
        wk_b = singles.tile([C - P, H * HS], F32R)
        wv_a = singles.tile([P, H * HS], BF16)
        wv_b = singles.tile([C - P, H * HS], BF16)
        nc.sync.dma_start(wq_a, wq[0:P, :])
        nc.sync.dma_start(wq_b, wq[P:C, :])
        nc.sync.dma_start(wk_a, wk[0:P, :])
        nc.sync.dma_start(wk_b, wk[P:C, :])
        nc.gpsimd.dma_start(wv_a, wv[0:P, :])
        nc.gpsimd.dma_start(wv_b, wv[P:C, :])
        for t0 in range(0, T, TCH):
            nc.sync.dma_start(xT_a[:, t0:t0 + TCH], xT[0:P, t0:t0 + TCH])
            nc.sync.dma_start(xT_b[:, t0:t0 + TCH], xT[P:C, t0:t0 + TCH])
            nc.gpsimd.dma_start(x16_a[:, t0:t0 + TCH], x16[0:P, t0:t0 + TCH])
            nc.gpsimd.dma_start(x16_b[:, t0:t0 + TCH], x16[P:C, t0:t0 + TCH])
        wpa_sb = singles.tile([P, C], BF16)
        wpb_sb = singles.tile([C - P + 1, C], BF16)
        id_sb = singles.tile([P, P], BF16)
        nc.sync.dma_start(wpa_sb, wpa[:, :])
        nc.sync.dma_start(wpb_sb, wpb[:, :])
        nc.sync.dma_start(id_sb, ident[:, :])

        qT_a = singles.tile([P, T], F32R)
        qT_b = singles.tile([C - P, T], F32R)
        kT_a = singles.tile([P, T], F32R)
        kT_b = singles.tile([C - P, T], F32R)
        otn_a = singles.tile([P, T], BF16)
        otn_b = singles.tile([C - P + 1, T], BF16)
        # ones row for the bias trick in the output projection
        nc.gpsimd.memset(otn_b[C - P:C - P + 1, :], 1.0)

        v33 = []
        for si in range(NS):
            v33.append(vpool.tile([P, H * 33], BF16, name=f"v33_{si}"))

        # ---------------- phase 1: qT, kT (fp32), v33 (bf16) ----------------
        with (
            tc.tile_pool(name="pq", bufs=2, space="PSUM") as pq,
            tc.tile_pool(name="pv", bufs=2, space="PSUM") as pv,
        ):
            for t0 in range(0, T, TCH):
                for name, wa, wb, dsta, dstb in (
                    ("q", wq_a, wq_b, qT_a, qT_b),
                    ("k", wk_a, wk_b, kT_a, kT_b),
                ):
                    for dlo, dsz, dst in ((0, P, dsta), (P, C - P, dstb)):
                        ps = pq.tile([P, TCH], F32, name="psq", tag="psq")
                        nc.tensor.matmul(
                            ps[0:dsz, :], wa[:, dlo:dlo + dsz],
                            xT_a[:, t0:t0 + TCH],
                            start=True, stop=False)
                        nc.tensor.matmul(
                            ps[0:dsz, :], wb[:, dlo:dlo + dsz],
                            xT_b[:, t0:t0 + TCH],
                            start=False, stop=True)
                        if name == "q":
                            nc.scalar.copy(dst[0:dsz, t0:t0 + TCH], ps[0:dsz, :])
                        else:
                            nc.vector.tensor_copy(
                                dst[0:dsz, t0:t0 + TCH], ps[0:dsz, :])
                for si in range(t0 // P, t0 // P + NJ):
                    s0 = si * P
                    ps = pv.tile([P, H * HS], F32, name="psv", tag="psv")
                    nc.tensor.matmul(ps, x16_a[:, s0:s0 + P],
                                     wv_a, start=True, stop=False)
                    nc.tensor.matmul(ps, x16_b[:, s0:s0 + P],
                                     wv_b, start=False, stop=True)
                    va_r = v33[si].rearrange("p (h e) -> p h e", h=H)
                    ps_r = ps.rearrange("p (h d) -> p h d", h=H)
                    nc.vector.tensor_copy(va_r[:, :, 0:HS], ps_r)
                    nc.gpsimd.memset(va_r[:, :, HS:HS + 1], 1.0)

        # ---------------- phase 2 ----------------
        def hsrc(h):
            if h < 4:
                return kT_a, qT_a, HS * h
            return kT_b, qT_b, HS * (h - 4)

        with (
            tc.tile_pool(name="pst", bufs=2, space="PSUM") as pst_pool,
            tc.tile_pool(name="pav", bufs=1, space="PSUM") as pav_pool,
            tc.tile_pool(name="pot", bufs=1, space="PSUM") as pot_pool,
            tc.tile_pool(name="py", bufs=1, space="PSUM") as py_pool,
        ):
            state = {}

            def emit_qkt(tci, si, p):
                tc0 = tci * TCH
                s0 = si * P
                stp = pst_pool.tile([P, 2 * TCH], F32, name="stp", tag="stp")
                for half in (0, 1):
                    h = 2 * p + half
                    kT_t, qT_t, pb = hsrc(h)
                    nc.tensor.matmul(
                        stp[:, half * TCH:(half + 1) * TCH],
                        kT_t[pb:pb + HS, s0:s0 + P],
                        qT_t[pb:pb + HS, tc0:tc0 + TCH],
                        start=True, stop=True, tile_position=(pb, 0))
                return stp

            def emit_exp(tci, si, p, stp):
                eng = _EXPPAT[si * 3 + p]
                ptp = ptp_pool.tile([P, 2 * TCH], BF16, name="ptp", tag="ptp")
                if eng == "A":
                    nc.scalar.activation(ptp, stp, Exp, scale=SCALE)
                else:
                    nc.vector.tensor_scalar(
                        ptp.bitcast(I16), stp, EXP_A, EXP_B,
                        mybir.AluOpType.mult, mybir.AluOpType.add)
                return ptp

            def emit_av(tci, si, ptps):
                # 24 matmuls: out[t,d] accumulated over si into 2 PSUM banks
                av = state["av"]
                for h in range(H):
                    bank, hl = divmod(h, 3)
                    ptp = ptps[h // 2]
                    half = h % 2
                    for j in range(NJ):
                        off = hl * 132 + j * 33
                        nc.tensor.matmul(
                            av[bank][:, off:off + 33],
                            ptp[:, half * TCH + j * P:half * TCH + (j + 1) * P],
                            v33[si][:, h * 33:(h + 1) * 33],
                            start=(si == 0 and hl == 0 and j == 0),
                            stop=(si == NS - 1 and hl == 2 and j == NJ - 1),
                            skip_group_check=True)

            def tail_norm(tci):
                av = state["av"]
                sa = stage_pool.tile([P, NJ * P], BF16, name="sa", tag="sa")
                sb = stage_pool.tile([P, NJ * (C - P)], BF16,
                                     name="sb", tag="sb")
                sa_v = sa.rearrange("p (j h e) -> p h j e", j=NJ, h=4)
                sb_v = sb.rearrange("p (j h e) -> p h j e", j=NJ, h=2)
                outs = []
                for bank in range(2):
                    rr = rr_pool.tile([P, 12], F32, name=f"rr{bank}",
                                      tag=f"rr{bank}")
                    av_v = av[bank][:, 0:396].rearrange(
                        "p (hl j e) -> p hl j e", hl=3, j=NJ)
                    rr_v = rr.rearrange("p (hl j e) -> p hl j e", hl=3, j=NJ)
                    nc.vector.reciprocal(rr_v, av_v[:, :, :, 32:33])
                    outs.append((av_v[:, :, :, 0:HS], rr_v))
                (o1, r1), (o2, r2) = outs
                for dst, src, rsrc in (
                    (sa_v[:, 0:3], o1, r1),
                    (sa_v[:, 3:4], o2[:, 0:1], r2[:, 0:1]),
                    (sb_v[:, 0:2], o2[:, 1:3], r2[:, 1:3]),
                ):
                    s_b, r_b = broadcast_tensor_aps(src, rsrc)
                    nc.vector.tensor_tensor(dst, s_b, r_b,
                                            mybir.AluOpType.mult)
                state["stage"] = (sa, sb)

            def tail_transpose(tci):
                sa, sb = state["stage"]
                pot = pot_pool.tile([P, 2 * NJ * P], BF16, name="pot",
                                    tag="pot")
                for j in range(NJ):
                    nc.tensor.transpose(
                        pot[:, j * 2 * P:j * 2 * P + P],
                        sa[:, j * P:(j + 1) * P], id_sb)
                    nc.tensor.transpose(
                        pot[0:C - P, j * 2 * P + P:(j + 1) * 2 * P],
                        sb[:, j * (C - P):(j + 1) * (C - P)], id_sb)
                state["pot"] = pot

            def tail_otcopy(tci):
                pot = state["pot"]
                tc0 = tci * TCH
                pot_v = pot.rearrange("p (j two t) -> p j two t", j=NJ, two=2)
                dst_a = otn_a[:, tc0:tc0 + TCH].rearrange(
                    "p (j t) -> p j t", j=NJ)
                dst_b = otn_b[0:C - P, tc0:tc0 + TCH].rearrange(
                    "p (j t) -> p j t", j=NJ)
                nc.vector.tensor_copy(dst_a, pot_v[:, :, 0, :])
                nc.vector.tensor_copy(dst_b, pot_v[0:C - P, :, 1, :])

            def tail_proj(tci, j):
                tt = tci * TCH + j * P
                py = py_pool.tile([P, C], F32, name="py", tag="py")
                nc.tensor.matmul(py, otn_a[:, tt:tt + P], wpa_sb,
                                 start=True, stop=False)
                nc.tensor.matmul(py, otn_b[:, tt:tt + P], wpb_sb,
                                 start=False, stop=True)
                ysb = ysb_pool.tile([P, C], F32, name="ysb", tag="ysb")
                nc.scalar.copy(ysb, py)
                nc.sync.dma_start(out[tt:tt + P, :], ysb)

            prev = None
            for tci in range(NT):
                av = [pav_pool.tile([P, TCH], F32, name=f"av{b}",
                                    tag=f"av{b}") for b in range(2)]
                if prev is not None:
                    tail_norm(prev)
                state["av"] = av
                pend = None
                for si in range(NS):
                    ptps = []
                    for p in range(3):
                        stp = emit_qkt(tci, si, p)
                        if p == 2:
                            if si == 1 and prev is not None:
                                tail_transpose(prev)
                            if pend is not None:
                                emit_av(tci, si - 1, pend)
                        ptps.append(emit_exp(tci, si, p, stp))
                    if prev is not None:
                        if si == 2:
                            tail_otcopy(prev)
                        elif 3 <= si <= 6:
                            tail_proj(prev, si - 3)
                    pend = ptps
                emit_av(tci, NS - 1, pend)
                prev = tci
            tail_norm(prev)
            tail_transpose(prev)
            tail_otcopy(prev)
            for j in range(NJ):
                tail_proj(prev, j)

    nc.compile()
    return nc


def _get_nc():
    if "nc" not in _CACHE:
        _CACHE["nc"] = build_nc()
    return _CACHE["nc"]


def make_in_maps(x, Wq, Wk, Wv, Wproj, bproj):
    bf = ml_dtypes.bfloat16
    x = np.asarray(x, np.float32)
    pack32 = lambda w: np.ascontiguousarray(
        np.transpose(np.asarray(w, np.float32), (1, 0, 2)).reshape(C, H * HS))
    wq_, wk_ = pack32(Wq), pack32(Wk)
    wv_ = pack32(Wv).astype(bf)
    wp = np.asarray(Wproj, np.float32)
    wpa_ = np.ascontiguousarray(wp[0:P, :]).astype(bf)
    wpb_ = np.concatenate(
        [wp[P:, :], np.asarray(bproj, np.float32).reshape(1, C)], axis=0
    ).astype(bf)
    ident = np.eye(P, dtype=np.float32).astype(bf)
    maps = []
    for i in range(B):
        xti = np.ascontiguousarray(x[i].T)
        maps.append({"xT": xti, "x16": xti.astype(bf), "wq": wq_, "wk": wk_,
                     "wv": wv_, "wpa": wpa_, "wpb": wpb_, "ident": ident})
    return maps


def run(inputs, trace=False, **kw):
    nc = _get_nc()
    in_maps = make_in_maps(**inputs)
    res = run_bass_kernel_spmd(nc, in_maps, core_ids=list(range(B)),
                               trace=trace, **kw)
    y = np.stack([np.asarray(res.results[i]["out"], np.float32)
                  for i in range(B)], axis=0)
    return y, res


def kernel(**inputs):
    y, _ = run(inputs, trace=False)
    return y
